# revision 4
# baseline (speedup 1.0000x reference)
"""Max-plus layer (y[b,i] = max_j(x[b,j] + a[i,j]) + bias[i]) on 8 TRN2 cores.

Strategy — tensor-parallel over out_features (64 per core), with a
three-engine PSUM pipeline per (feature, batch-tile) pair so the DVE
only does the irreducible max-reduce pass:

  1. host: fold bias into a (a' = a + bias), split a' EXACTLY into
     three bf16 planes (hi + mid + lo == a' bitwise in fp32).
  2. ScalarE copies the x batch-tile [128, 512] into a PSUM bank
     (bit-exact fp32 copy; the bank's has_written bits were set once by
     a zero bf16 matmul, so later matmuls accumulate instead of
     overwriting — verified on HW).
  3. TensorE adds the broadcast a'-row with three accumulating bf16
     matmuls (ones^T[1,128] @ a_plane[1,512]); exact in fp32 PSUM.
  4. VectorE max-reduces the bank into one y column.

All engines run concurrently across the 8 independent bank chains;
the kernel is DVE-reduce-bound (~0.7us x 512 pairs per core).
"""

import sys

sys.path.insert(0, "/opt/trn_rl_repo")

import ml_dtypes
import numpy as np

import concourse.bass as bass
import concourse.mybir as mybir
import concourse.tile as tile
from concourse import bacc
from concourse.bass_utils import run_bass_kernel_spmd

F32 = mybir.dt.float32
BF16 = mybir.dt.bfloat16
AX = None  # set lazily

B = 1024  # batch
J = 512  # in_features
O = 512  # out_features
N_CORES = 8
O_SH = O // N_CORES  # 64 output features per core
NBT = B // 128  # 8 batch tiles
ROWS_PER_PART = 32  # a'-rows stored per SBUF partition (2 partitions)
ROW_BYTES = 3 * J  # hi|mid|lo concatenated per row (in elements)

TRACE = False
LAST_RESULTS = None
_nc_cache = None


def _build_bass():
    nc = bacc.Bacc("TRN2", target_bir_lowering=False, debug=False, num_devices=N_CORES)
    x_t = nc.dram_tensor("x", [B, J], F32, kind="ExternalInput")
    a3_t = nc.dram_tensor(
        "a3", [2, ROWS_PER_PART * ROW_BYTES], BF16, kind="ExternalInput"
    )
    cs_t = nc.dram_tensor("consts", [2, 128 + J], BF16, kind="ExternalInput")
    y_t = nc.dram_tensor("y", [B, O_SH], F32, kind="ExternalOutput")

    with tile.TileContext(nc) as tc:
        with (
            tc.tile_pool(name="sb", bufs=1) as sb,
            tc.tile_pool(name="ps", bufs=1, space="PSUM") as ps,
        ):
            x_sb = [
                sb.tile([128, J], F32, tag=f"x{t}", name=f"x_sb{t}")
                for t in range(NBT)
            ]
            for t in range(NBT):
                nc.sync.dma_start(x_sb[t][:], x_t.ap()[t * 128 : (t + 1) * 128, :])
            # payload rows live at base partitions 0 and 32 (the only
            # slice bases bass allows for matmul operands)
            a3_sb = sb.tile([33, ROWS_PER_PART * ROW_BYTES], BF16)
            nc.sync.dma_start(a3_sb[0:1, :], a3_t.ap()[0:1, :])
            nc.sync.dma_start(a3_sb[32:33, :], a3_t.ap()[1:2, :])
            cs_sb = sb.tile([33, 128 + J], BF16)
            nc.sync.dma_start(cs_sb[0:1, :], cs_t.ap()[0:1, :])
            nc.sync.dma_start(cs_sb[32:33, :], cs_t.ap()[1:2, :])

            banks = [
                ps.tile([128, J], F32, tag=f"bank{t}", name=f"bank{t}")
                for t in range(NBT)
            ]
            y_sb = [
                sb.tile([128, O_SH], F32, tag=f"y{t}", name=f"y_sb{t}")
                for t in range(NBT)
            ]

            ones0 = cs_sb[0:1, 0:128]
            zeros0 = cs_sb[0:1, 128 : 128 + J]
            # one-time: set has_written for every bank
            for t in range(NBT):
                nc.tensor.matmul(
                    banks[t][:], lhsT=ones0, rhs=zeros0, start=True, stop=False
                )

            for i in range(O_SH):
                part = (i // ROWS_PER_PART) * 32
                base = (i % ROWS_PER_PART) * ROW_BYTES
                lhsT = cs_sb[part : part + 1, 0:128]
                for t in range(NBT):
                    nc.scalar.copy(banks[t][:], x_sb[t][:])
                    for k in range(3):
                        nc.tensor.matmul(
                            banks[t][:],
                            lhsT=lhsT,
                            rhs=a3_sb[part : part + 1, base + k * J : base + (k + 1) * J],
                            start=False,
                            stop=(k == 2),
                        )
                    nc.vector.tensor_reduce(
                        y_sb[t][:, i : i + 1],
                        banks[t][:],
                        mybir.AxisListType.X,
                        mybir.AluOpType.max,
                    )

            for t in range(NBT):
                nc.sync.dma_start(y_t.ap()[t * 128 : (t + 1) * 128, :], y_sb[t][:])
    nc.compile()
    return nc


def _prep_inputs(x, a, bias):
    """Host-side prep: fold bias, exact 3-way bf16 split, per-core shards."""
    a_p = (a.astype(np.float64) + bias.astype(np.float64)[:, None]).astype(np.float32)
    a_hi = a_p.astype(ml_dtypes.bfloat16)
    r1 = a_p - a_hi.astype(np.float32)
    a_mid = r1.astype(ml_dtypes.bfloat16)
    r2 = r1 - a_mid.astype(np.float32)
    a_lo = r2.astype(ml_dtypes.bfloat16)
    assert np.all(r2 - a_lo.astype(np.float32) == 0.0), "bf16 split not exact"

    # [O, 3*J] rows: hi|mid|lo
    rows = np.concatenate([a_hi, a_mid, a_lo], axis=1)

    consts = np.zeros((2, 128 + J), ml_dtypes.bfloat16)
    consts[:, 0:128] = 1.0

    in_maps = []
    for c in range(N_CORES):
        shard = rows[c * O_SH : (c + 1) * O_SH]  # [64, 1536]
        a3 = np.ascontiguousarray(
            shard.reshape(2, ROWS_PER_PART * ROW_BYTES)
        )
        in_maps.append({"x": x, "a3": a3, "consts": consts})
    return in_maps


def kernel(x, a, bias):
    global _nc_cache, LAST_RESULTS
    x = np.ascontiguousarray(np.asarray(x, dtype=np.float32))
    a = np.asarray(a, dtype=np.float32)
    bias = np.asarray(bias, dtype=np.float32)
    assert x.shape == (B, J) and a.shape == (O, J) and bias.shape == (O,)

    if _nc_cache is None:
        _nc_cache = _build_bass()
    nc = _nc_cache

    in_maps = _prep_inputs(x, a, bias)
    res = run_bass_kernel_spmd(
        nc, in_maps, core_ids=list(range(N_CORES)), trace=TRACE
    )
    LAST_RESULTS = res
    y = np.concatenate([res.results[c]["y"] for c in range(N_CORES)], axis=1)
    return y
